# revision 9
# baseline (speedup 1.0000x reference)
"""GQA kernel for Trainium2: B=2,T=2048,E=2048,G=4,QPG=4,D=128, causal + sinusoidal PE.

Sharding: one core per (batch, kv-group) pair = 2*4 = 8 cores.
Each core computes q/k/v projections for its group, attention for its 4 query
heads, and a partial output projection (its group's 512 columns of wo);
partials are summed on the host.

v2 layout strategy (per core), all matmul operands bf16:
  - projections: weight tile stationary, x^T streams; per (tb, chain) one
    512-col psum chain over 16 e-tiles. pe^T+bias is folded host-side into
    petq/petk so each drain is a single DVE add.
  - scores S^T[tk, tq]: kt tile stationary, q^T streams 512 cols; exp'd on
    ACT into bf16 P^T tiles (no max subtraction; scores bounded).
  - PV is flipped vs v1: V tile [tk, d] is stationary and P^T streams 512
    cols, producing O^T[d, tq] directly in psum (no output transposes).
  - softmax denominators: R = sum_tk P^T accumulated on DVE, then gpsimd
    partition_all_reduce replicates the column sums to all partitions;
    reciprocal multiplies O^T elementwise during the psum drain.
  - phases are software-pipelined: the scalar engine's exp (~570ns/tile) is
    ~2.6x slower than a score matmul, so projection half-chains of block
    tb+1 and output-projection units of block qb-1 are interleaved into the
    scores stretch one unit per accumulated lag quantum.
"""
import sys

sys.path.insert(0, "/opt/trn_rl_repo")

import math
import numpy as np

B, T, E = 2, 2048, 2048
G, QPG, D = 4, 4, 128
NQ = QPG * D          # 512 q columns per group
NKV = 2 * D           # 256 kv columns per group
TT = T // 128         # 16 t-tiles
TB = T // 512         # 4 t-blocks
NE = E // 128         # 16 e-tiles
ISD = 1.0 / math.sqrt(D)

_compiled = None


def _build():
    from collections import deque
    from concourse import bacc, tile, mybir, bass_isa

    f32 = mybir.dt.float32
    bf16 = mybir.dt.bfloat16
    ADD = mybir.AluOpType.add
    MULT = mybir.AluOpType.mult
    EXP = mybir.ActivationFunctionType.Exp
    IDENT = mybir.ActivationFunctionType.Identity
    RADD = bass_isa.ReduceOp.add

    nc = bacc.Bacc("TRN2", target_bir_lowering=False, debug=False, num_devices=8)

    xt_d = nc.dram_tensor("xt", [128, NE, T], bf16, kind="ExternalInput")    # x^T packed
    wqkv_d = nc.dram_tensor("wqkv", [128, NE, NQ + NKV], bf16, kind="ExternalInput")
    wo_d = nc.dram_tensor("wo", [NQ, E], bf16, kind="ExternalInput")        # group slice
    pet_d = nc.dram_tensor("pet", [D, T], bf16, kind="ExternalInput")         # pe^T
    bq_d = nc.dram_tensor("bq", [D, QPG], f32, kind="ExternalInput")
    bk_d = nc.dram_tensor("bk", [D, 1], f32, kind="ExternalInput")
    bv_d = nc.dram_tensor("bv", [D, 1], f32, kind="ExternalInput")
    msk_d = nc.dram_tensor("msk", [4, 128, 512], bf16, kind="ExternalInput")
    idb_d = nc.dram_tensor("idb", [128, 128], bf16, kind="ExternalInput")
    out_d = nc.dram_tensor("out", [T, E], bf16, kind="ExternalOutput")

    with tile.TileContext(nc) as tc:
        with (
            tc.tile_pool(name="main", bufs=1) as pp,
            tc.tile_pool(name="ps", bufs=1, space="PSUM") as ps,
        ):
            # ---- persistent constants (gpsimd DMA queue; phase-1 stream is
            # on sync so these never delay the first matmuls) ----
            bv = pp.tile([D, 1], f32)
            nc.gpsimd.dma_start(bv[:], bv_d[:])
            bq = pp.tile([D, QPG], f32)
            nc.gpsimd.dma_start(bq[:], bq_d[:])
            bk = pp.tile([D, 1], f32)
            nc.gpsimd.dma_start(bk[:], bk_d[:])
            idb = pp.tile([128, 128], bf16)
            nc.gpsimd.dma_start(idb[:], idb_d[:])
            pet = pp.tile([D, T], bf16)
            nc.scalar.dma_start(pet[:], pet_d[:])
            msk = [pp.tile([128, 512], bf16, name=f"msk{j}", tag=f"msk{j}") for j in range(4)]
            wo_sb = [pp.tile([128, E], bf16, name=f"wo{h}", tag=f"wo{h}") for h in range(QPG)]

            # ---- persistent activations ----
            qt = [pp.tile([128, T], bf16, name=f"qt{h}", tag=f"qt{h}") for h in range(QPG)]
            kt = pp.tile([128, T], bf16)
            at = [pp.tile([128, T], bf16, name=f"at{h}", tag=f"at{h}") for h in range(QPG)]
            vxall = pp.tile([128, T], bf16, name="vxall", tag="vxall")

            # ---- phase-1 weights + x^T stream (4-e-chunk DMAs, split queues) ----
            wqkv_sb = [pp.tile([128, 4, NQ + NKV], bf16, name=f"wqkv{g}", tag=f"wqkv{g}")
                       for g in range(4)]
            xt_t = [[None] * 4 for _ in range(TB)]

            def emit_xt_dma(tb):
                ts = slice(tb * 512, (tb + 1) * 512)
                for g in range(4):
                    xt_t[tb][g] = pp.tile([128, 4, 512], bf16, name="xt", tag="xt", bufs=6)
                    nc.sync.dma_start(xt_t[tb][g][:], xt_d[:, 4 * g:4 * g + 4, ts])

            # block-0 feed is latency-critical: per-e DMAs spread over four
            # otherwise-idle engine queues so the first chains start immediately
            for g in range(4):
                xt_t[0][g] = pp.tile([128, 4, 512], bf16, name="xt", tag="xt", bufs=6)
            qs3 = [nc.sync, nc.scalar, nc.gpsimd]
            for e in range(NE):
                g, i = divmod(e, 4)
                qs3[(2 * e) % 3].dma_start(wqkv_sb[g][:, i, :], wqkv_d[:, e, :])
                qs3[(2 * e + 1) % 3].dma_start(xt_t[0][g][:, i, :], xt_d[:, e, 0:512])
            for j in range(4):
                nc.gpsimd.dma_start(msk[j][:], msk_d[j])
            for h in range(QPG):
                nc.scalar.dma_start(wo_sb[h][:], wo_d[h * 128:(h + 1) * 128, :])

            # ---- unit generators; units are (cost_ns, fn) ----
            def proj_units(tb, c):
                # c: 0-3 = q head c, 4 = k, 5 = v; two half-chain units
                ts = slice(tb * 512, (tb + 1) * 512)
                state = {}

                def wsel(e):
                    g, i = divmod(e, 4)
                    if c < 4:
                        return wqkv_sb[g][:, i, c * 128:(c + 1) * 128]
                    if c == 4:
                        return wqkv_sb[g][:, i, NQ:NQ + 128]
                    return wqkv_sb[g][:, i, NQ + 128:NQ + 256]

                def half_a():
                    ps_t = ps.tile([128, 512], f32, name="big", tag="big", bufs=3)
                    state["ps"] = ps_t
                    for e in range(8):
                        nc.tensor.matmul(ps_t[:], wsel(e), xt_t[tb][e // 4][:, e % 4, :],
                                         start=(e == 0), stop=False)

                def half_b():
                    ps_t = state["ps"]
                    for e in range(8, NE):
                        nc.tensor.matmul(ps_t[:], wsel(e), xt_t[tb][e // 4][:, e % 4, :],
                                         start=False, stop=(e == NE - 1))
                    if c < 4:
                        nc.vector.tensor_tensor(ps_t[:], ps_t[:],
                                                bq[:, c:c + 1].to_broadcast([128, 512]), ADD)
                        nc.vector.tensor_tensor(qt[c][:, ts], ps_t[:], pet[:, ts], ADD)
                    elif c == 4:
                        nc.vector.tensor_tensor(ps_t[:], ps_t[:],
                                                bk[:].to_broadcast([128, 512]), ADD)
                        nc.vector.tensor_tensor(kt[:, ts], ps_t[:], pet[:, ts], ADD)
                    else:
                        vtb = pp.tile([128, 512], bf16, name="vtb", tag="vtb", bufs=2)
                        nc.vector.tensor_tensor(vtb[:], ps_t[:],
                                                bv[:].to_broadcast([128, 512]), ADD)
                        vtp = ps.tile([128, 512], bf16, name="vtp", tag="vtp", bufs=1)
                        for i in range(4):
                            nc.tensor.transpose(vtp[:, i * 128:(i + 1) * 128],
                                                vtb[:, i * 128:(i + 1) * 128], idb[:])
                        nc.vector.tensor_copy(vxall[:, ts], vtp[:])

                return [(1800, half_a), (2100, half_b)]

            def oproj_units(ti):
                state = {}

                def alloc():
                    state["o"] = pp.tile([128, E], bf16, name="osb", tag="osb", bufs=2)

                units = [(0, alloc)]
                for eo in range(4):
                    def one(eo=eo):
                        w_ps = ps.tile([128, 512], f32, name="big", tag="big", bufs=3)
                        for h in range(QPG):
                            nc.tensor.matmul(
                                w_ps[:], at[h][:, ti * 128:(ti + 1) * 128],
                                wo_sb[h][:, eo * 512:(eo + 1) * 512],
                                start=(h == 0), stop=(h == QPG - 1),
                            )
                        dst = state["o"][:, eo * 512:(eo + 1) * 512]
                        if eo % 2 == 0:
                            nc.vector.tensor_copy(dst, w_ps[:])
                        else:
                            nc.scalar.activation(dst, w_ps[:], IDENT, scale=1.0)
                    units.append((1000, one))

                def store():
                    nc.sync.dma_start(out_d[ti * 128:(ti + 1) * 128, :], state["o"][:])
                units.append((0, store))
                return units

            # two filler queues: proj has a deadline (before next block's
            # scores), oproj is slack-filled
            fill_proj = deque()
            fill_oproj = deque()
            lag = [0]

            def absorb(extra):
                lag[0] += extra
                while lag[0] > 0 and (fill_proj or fill_oproj):
                    q = fill_proj if fill_proj else fill_oproj
                    cost, fn = q.popleft()
                    fn()
                    lag[0] -= cost

            # ---- phase-2 per block qb, heads in pairs ----
            def phase2_pair(qb, pair):
                qs = slice(qb * 512, (qb + 1) * 512)
                nkt = 4 * qb + 4
                pts = {h: [] for h in pair}
                R = {h: pp.tile([128, 512], bf16, name=f"R{h}", tag=f"R{h}", bufs=2)
                     for h in pair}
                for tk in range(nkt):
                    for h in pair:
                        s_ps = ps.tile([128, 512], f32, name="s", tag="s", bufs=2)
                        nc.tensor.matmul(s_ps[:], kt[:, tk * 128:(tk + 1) * 128],
                                         qt[h][:, qs], start=True, stop=True)
                        p_t = pp.tile([128, 512], bf16, name="pt", tag="pt", bufs=34)
                        nc.scalar.activation(p_t[:], s_ps[:], EXP, scale=ISD)
                        j = tk - 4 * qb
                        if j >= 0:
                            nc.vector.tensor_tensor(p_t[:], p_t[:], msk[j][:], MULT)
                        pts[h].append(p_t)
                        absorb(460)
                o_ps = {h: ps.tile([128, 512], f32, name="o", tag="o", bufs=2)
                        for h in pair}
                for tk in range(nkt):
                    for h in pair:
                        nc.tensor.matmul(o_ps[h][:], vxall[:, tk * 128:(tk + 1) * 128],
                                         pts[h][tk][:],
                                         start=(tk == 0), stop=(tk == nkt - 1))
                        # R accumulation rides the PV stretch, when DVE is
                        # otherwise idle; it only gates the allreduce below
                        if tk == 0:
                            nc.vector.tensor_copy(R[h][:], pts[h][0][:])
                        else:
                            nc.vector.tensor_tensor(R[h][:], R[h][:], pts[h][tk][:], ADD)
                rcp = {}
                for h in pair:
                    allR = pp.tile([128, 512], f32, name="allR", tag="allR", bufs=2)
                    nc.gpsimd.partition_all_reduce(allR[:], R[h][:], 128, RADD)
                    r = pp.tile([128, 512], f32, name=f"rcp{h}", tag=f"rcp{h}", bufs=2)
                    nc.vector.reciprocal_approx_fast(r[:], allR[:])
                    rcp[h] = r
                for h in pair:
                    nc.vector.tensor_tensor(at[h][:, qs], o_ps[h][:], rcp[h][:], MULT)

            # ---- drive ----
            for c in range(6):
                for _, fn in proj_units(0, c):
                    fn()

            for qb in range(TB):
                if qb + 1 < TB:
                    emit_xt_dma(qb + 1)
                    for c in range(6):
                        fill_proj.extend(proj_units(qb + 1, c))
                phase2_pair(qb, (0, 1))
                phase2_pair(qb, (2, 3))
                # proj chains for tb=qb+1 must land before scores(qb+1)
                while fill_proj:
                    fill_proj.popleft()[1]()
                for ti in range(4 * qb, 4 * qb + 4):
                    fill_oproj.extend(oproj_units(ti))
            while fill_oproj:
                fill_oproj.popleft()[1]()

    nc.compile()
    return nc


def _get_compiled():
    global _compiled
    if _compiled is None:
        _compiled = _build()
    return _compiled


def _host_inputs(x, wq, bq, wkv, bkv, wo):
    import jax.numpy as jnp

    def to_bf16(a):
        return np.asarray(jnp.asarray(a, dtype=jnp.bfloat16))

    pos = np.arange(T, dtype=np.float32)[:, None]
    i = np.arange(0, D, 2, dtype=np.float32)
    inv = np.exp(-(np.log(10000.0) * i / D))
    ang = pos * inv
    pe = np.zeros((T, D), np.float32)
    pe[:, 0::2] = np.sin(ang)
    pe[:, 1::2] = np.cos(ang)
    pet = np.ascontiguousarray(pe.T)                       # [D, T]

    # causal masks for the 4 diagonal tiles of a 512-wide tq block:
    # mask_j[p, c] = 1 if c >= 128*j + p
    c = np.arange(512)[None, :]
    p = np.arange(128)[:, None]
    msk = to_bf16(np.stack([(c >= 128 * j + p) for j in range(4)]).astype(np.float32))
    idb = to_bf16(np.eye(128, dtype=np.float32))

    xts = [to_bf16(np.ascontiguousarray(
        x[b].T.reshape(NE, 128, T).transpose(1, 0, 2))) for b in range(B)]
    in_maps = []
    for core in range(8):
        b, g = divmod(core, G)
        bq_g = bq[g * NQ:(g + 1) * NQ].reshape(QPG, D)     # [h, d]
        wqkv = np.concatenate(
            [wq[:, g * NQ:(g + 1) * NQ], wkv[:, g * NKV:(g + 1) * NKV]], axis=1)
        in_maps.append({
            "xt": xts[b],
            "wqkv": to_bf16(np.ascontiguousarray(
                wqkv.reshape(NE, 128, NQ + NKV).transpose(1, 0, 2))),
            "wo": to_bf16(wo[g * NQ:(g + 1) * NQ, :]),
            "pet": to_bf16(pet),
            "bq": np.ascontiguousarray(bq_g.T).astype(np.float32),
            "bk": np.ascontiguousarray(
                bkv[g * NKV:g * NKV + D].reshape(D, 1)).astype(np.float32),
            "bv": np.ascontiguousarray(
                bkv[g * NKV + D:(g + 1) * NKV].reshape(D, 1)).astype(np.float32),
            "msk": msk,
            "idb": idb,
        })
    return in_maps


def run(x, wq, bq, wkv, bkv, wo, trace=False):
    from concourse.bass_utils import run_bass_kernel_spmd

    nc = _get_compiled()
    in_maps = _host_inputs(
        np.asarray(x, np.float32), np.asarray(wq, np.float32),
        np.asarray(bq, np.float32), np.asarray(wkv, np.float32),
        np.asarray(bkv, np.float32), np.asarray(wo, np.float32),
    )
    res = run_bass_kernel_spmd(nc, in_maps, core_ids=list(range(8)), trace=trace)
    out = np.zeros((B, T, E), np.float32)
    for core in range(8):
        b = core // G
        out[b] += np.asarray(res.results[core]["out"], dtype=np.float32)
    return out, res


def kernel(x, wq, bq, wkv, bkv, wo):
    out, _ = run(x, wq, bq, wkv, bkv, wo, trace=False)
    return out


# revision 10
# speedup vs baseline: 1.0022x; 1.0022x over previous
"""GQA kernel for Trainium2: B=2,T=2048,E=2048,G=4,QPG=4,D=128, causal + sinusoidal PE.

Sharding: one core per (batch, kv-group) pair = 2*4 = 8 cores.
Each core computes q/k/v projections for its group, attention for its 4 query
heads, and a partial output projection (its group's 512 columns of wo);
partials are summed on the host.

v2 layout strategy (per core), all matmul operands bf16:
  - projections: weight tile stationary, x^T streams; per (tb, chain) one
    512-col psum chain over 16 e-tiles. pe^T+bias is folded host-side into
    petq/petk so each drain is a single DVE add.
  - scores S^T[tk, tq]: kt tile stationary, q^T streams 512 cols; exp'd on
    ACT into bf16 P^T tiles (no max subtraction; scores bounded).
  - PV is flipped vs v1: V tile [tk, d] is stationary and P^T streams 512
    cols, producing O^T[d, tq] directly in psum (no output transposes).
  - softmax denominators: R = sum_tk P^T accumulated on DVE, then gpsimd
    partition_all_reduce replicates the column sums to all partitions;
    reciprocal multiplies O^T elementwise during the psum drain.
  - phases are software-pipelined: the scalar engine's exp (~570ns/tile) is
    ~2.6x slower than a score matmul, so projection half-chains of block
    tb+1 and output-projection units of block qb-1 are interleaved into the
    scores stretch one unit per accumulated lag quantum.
"""
import sys

sys.path.insert(0, "/opt/trn_rl_repo")

import math
import numpy as np

B, T, E = 2, 2048, 2048
G, QPG, D = 4, 4, 128
NQ = QPG * D          # 512 q columns per group
NKV = 2 * D           # 256 kv columns per group
TT = T // 128         # 16 t-tiles
TB = T // 512         # 4 t-blocks
NE = E // 128         # 16 e-tiles
ISD = 1.0 / math.sqrt(D)

_compiled = None


def _build():
    from collections import deque
    from concourse import bacc, tile, mybir, bass_isa

    f32 = mybir.dt.float32
    bf16 = mybir.dt.bfloat16
    ADD = mybir.AluOpType.add
    MULT = mybir.AluOpType.mult
    EXP = mybir.ActivationFunctionType.Exp
    IDENT = mybir.ActivationFunctionType.Identity
    RADD = bass_isa.ReduceOp.add

    nc = bacc.Bacc("TRN2", target_bir_lowering=False, debug=False, num_devices=8)

    xt_d = nc.dram_tensor("xt", [128, NE, T], bf16, kind="ExternalInput")    # x^T packed
    wqkv_d = nc.dram_tensor("wqkv", [128, NE, NQ + NKV], bf16, kind="ExternalInput")
    wo_d = nc.dram_tensor("wo", [NQ, E], bf16, kind="ExternalInput")        # group slice
    pet_d = nc.dram_tensor("pet", [D, T], bf16, kind="ExternalInput")         # pe^T
    bq_d = nc.dram_tensor("bq", [D, QPG], f32, kind="ExternalInput")
    bk_d = nc.dram_tensor("bk", [D, 1], f32, kind="ExternalInput")
    bv_d = nc.dram_tensor("bv", [D, 1], f32, kind="ExternalInput")
    msk_d = nc.dram_tensor("msk", [4, 128, 512], bf16, kind="ExternalInput")
    idb_d = nc.dram_tensor("idb", [128, 128], bf16, kind="ExternalInput")
    out_d = nc.dram_tensor("out", [T, E], bf16, kind="ExternalOutput")

    with tile.TileContext(nc) as tc:
        with (
            tc.tile_pool(name="main", bufs=1) as pp,
            tc.tile_pool(name="ps", bufs=1, space="PSUM") as ps,
        ):
            # ---- persistent constants (gpsimd DMA queue; phase-1 stream is
            # on sync so these never delay the first matmuls) ----
            bv = pp.tile([D, 1], f32)
            nc.gpsimd.dma_start(bv[:], bv_d[:])
            bq = pp.tile([D, QPG], f32)
            nc.gpsimd.dma_start(bq[:], bq_d[:])
            bk = pp.tile([D, 1], f32)
            nc.gpsimd.dma_start(bk[:], bk_d[:])
            idb = pp.tile([128, 128], bf16)
            nc.gpsimd.dma_start(idb[:], idb_d[:])
            pet = pp.tile([D, T], bf16)
            msk = [pp.tile([128, 512], bf16, name=f"msk{j}", tag=f"msk{j}") for j in range(4)]
            wo_sb = [pp.tile([128, E], bf16, name=f"wo{h}", tag=f"wo{h}") for h in range(QPG)]

            # ---- persistent activations ----
            qt = [pp.tile([128, T], bf16, name=f"qt{h}", tag=f"qt{h}") for h in range(QPG)]
            kt = pp.tile([128, T], bf16)
            at = [pp.tile([128, T], bf16, name=f"at{h}", tag=f"at{h}") for h in range(QPG)]
            vxall = pp.tile([128, T], bf16, name="vxall", tag="vxall")

            # ---- phase-1 weights + x^T stream (4-e-chunk DMAs, split queues) ----
            wqkv_sb = [pp.tile([128, 4, NQ + NKV], bf16, name=f"wqkv{g}", tag=f"wqkv{g}")
                       for g in range(4)]
            xt_t = [[None] * 4 for _ in range(TB)]

            def emit_xt_dma(tb):
                ts = slice(tb * 512, (tb + 1) * 512)
                for g in range(4):
                    xt_t[tb][g] = pp.tile([128, 4, 512], bf16, name="xt", tag="xt", bufs=6)
                    nc.sync.dma_start(xt_t[tb][g][:], xt_d[:, 4 * g:4 * g + 4, ts])

            # block-0 feed is latency-critical: per-e DMAs spread over four
            # otherwise-idle engine queues so the first chains start immediately
            for g in range(4):
                xt_t[0][g] = pp.tile([128, 4, 512], bf16, name="xt", tag="xt", bufs=6)
            qs3 = [nc.sync, nc.scalar, nc.gpsimd]
            for e in range(NE):
                g, i = divmod(e, 4)
                qs3[(2 * e) % 3].dma_start(wqkv_sb[g][:, i, :], wqkv_d[:, e, :])
                qs3[(2 * e + 1) % 3].dma_start(xt_t[0][g][:, i, :], xt_d[:, e, 0:512])
            nc.scalar.dma_start(pet[:], pet_d[:])
            for j in range(4):
                nc.gpsimd.dma_start(msk[j][:], msk_d[j])
            for h in range(QPG):
                nc.scalar.dma_start(wo_sb[h][:], wo_d[h * 128:(h + 1) * 128, :])

            # ---- unit generators; units are (cost_ns, fn) ----
            def proj_units(tb, c):
                # c: 0-3 = q head c, 4 = k, 5 = v; two half-chain units
                ts = slice(tb * 512, (tb + 1) * 512)
                state = {}

                def wsel(e):
                    g, i = divmod(e, 4)
                    if c < 4:
                        return wqkv_sb[g][:, i, c * 128:(c + 1) * 128]
                    if c == 4:
                        return wqkv_sb[g][:, i, NQ:NQ + 128]
                    return wqkv_sb[g][:, i, NQ + 128:NQ + 256]

                def half_a():
                    ps_t = ps.tile([128, 512], f32, name="big", tag="big", bufs=3)
                    state["ps"] = ps_t
                    for e in range(8):
                        nc.tensor.matmul(ps_t[:], wsel(e), xt_t[tb][e // 4][:, e % 4, :],
                                         start=(e == 0), stop=False)

                def half_b():
                    ps_t = state["ps"]
                    for e in range(8, NE):
                        nc.tensor.matmul(ps_t[:], wsel(e), xt_t[tb][e // 4][:, e % 4, :],
                                         start=False, stop=(e == NE - 1))

                def drain():
                    # emitted one unit late: ps_t is already complete when this
                    # lands in the DVE queue, so masks/exps behind it never block
                    ps_t = state["ps"]
                    if c < 4:
                        nc.vector.tensor_tensor(ps_t[:], ps_t[:],
                                                bq[:, c:c + 1].to_broadcast([128, 512]), ADD)
                        nc.vector.tensor_tensor(qt[c][:, ts], ps_t[:], pet[:, ts], ADD)
                    elif c == 4:
                        nc.vector.tensor_tensor(ps_t[:], ps_t[:],
                                                bk[:].to_broadcast([128, 512]), ADD)
                        nc.vector.tensor_tensor(kt[:, ts], ps_t[:], pet[:, ts], ADD)
                    else:
                        vtb = pp.tile([128, 512], bf16, name="vtb", tag="vtb", bufs=2)
                        nc.vector.tensor_tensor(vtb[:], ps_t[:],
                                                bv[:].to_broadcast([128, 512]), ADD)
                        vtp = ps.tile([128, 512], bf16, name="vtp", tag="vtp", bufs=1)
                        for i in range(4):
                            nc.tensor.transpose(vtp[:, i * 128:(i + 1) * 128],
                                                vtb[:, i * 128:(i + 1) * 128], idb[:])
                        nc.vector.tensor_copy(vxall[:, ts], vtp[:])

                return [(1800, half_a), (1800, half_b), (300, drain)]

            def oproj_units(ti):
                state = {}

                def alloc():
                    state["o"] = pp.tile([128, E], bf16, name="osb", tag="osb", bufs=2)

                def drain_prev():
                    # deferred drain: the stashed w_ps finished on PE during the
                    # previous unit, so this never blocks the DVE/ACT queue
                    eo, w_ps = state["pend"]
                    dst = state["o"][:, eo * 512:(eo + 1) * 512]
                    if eo % 2 == 0:
                        nc.vector.tensor_copy(dst, w_ps[:])
                    else:
                        nc.scalar.activation(dst, w_ps[:], IDENT, scale=1.0)

                units = [(0, alloc)]
                for eo in range(4):
                    def one(eo=eo):
                        if eo > 0:
                            drain_prev()
                        w_ps = ps.tile([128, 512], f32, name="big", tag="big", bufs=3)
                        for h in range(QPG):
                            nc.tensor.matmul(
                                w_ps[:], at[h][:, ti * 128:(ti + 1) * 128],
                                wo_sb[h][:, eo * 512:(eo + 1) * 512],
                                start=(h == 0), stop=(h == QPG - 1),
                            )
                        state["pend"] = (eo, w_ps)
                    units.append((1000, one))

                def store():
                    drain_prev()
                    nc.sync.dma_start(out_d[ti * 128:(ti + 1) * 128, :], state["o"][:])
                units.append((300, store))
                return units

            # two filler queues: proj has a deadline (before next block's
            # scores), oproj is slack-filled
            fill_proj = deque()
            fill_oproj = deque()
            lag = [0]

            def absorb(extra):
                lag[0] += extra
                while lag[0] > 0 and (fill_proj or fill_oproj):
                    q = fill_proj if fill_proj else fill_oproj
                    cost, fn = q.popleft()
                    fn()
                    lag[0] -= cost

            # ---- phase-2 per block qb, heads in pairs ----
            def phase2_pair(qb, pair):
                qs = slice(qb * 512, (qb + 1) * 512)
                nkt = 4 * qb + 4
                pts = {h: [] for h in pair}
                R = {h: pp.tile([128, 512], bf16, name=f"R{h}", tag=f"R{h}", bufs=2)
                     for h in pair}
                for tk in range(nkt):
                    for h in pair:
                        s_ps = ps.tile([128, 512], f32, name="s", tag="s", bufs=2)
                        nc.tensor.matmul(s_ps[:], kt[:, tk * 128:(tk + 1) * 128],
                                         qt[h][:, qs], start=True, stop=True)
                        p_t = pp.tile([128, 512], bf16, name="pt", tag="pt", bufs=40)
                        nc.scalar.activation(p_t[:], s_ps[:], EXP, scale=ISD)
                        j = tk - 4 * qb
                        if j >= 0:
                            nc.vector.tensor_tensor(p_t[:], p_t[:], msk[j][:], MULT)
                        pts[h].append(p_t)
                        absorb(460)
                o_ps = {h: ps.tile([128, 512], f32, name="o", tag="o", bufs=2)
                        for h in pair}
                for tk in range(nkt):
                    for h in pair:
                        nc.tensor.matmul(o_ps[h][:], vxall[:, tk * 128:(tk + 1) * 128],
                                         pts[h][tk][:],
                                         start=(tk == 0), stop=(tk == nkt - 1))
                        # R accumulation rides the PV stretch, when DVE is
                        # otherwise idle; it only gates the allreduce below
                        if tk == 0:
                            nc.vector.tensor_copy(R[h][:], pts[h][0][:])
                        else:
                            nc.vector.tensor_tensor(R[h][:], R[h][:], pts[h][tk][:], ADD)
                rcp = {}
                for h in pair:
                    allR = pp.tile([128, 512], f32, name="allR", tag="allR", bufs=2)
                    nc.gpsimd.partition_all_reduce(allR[:], R[h][:], 128, RADD)
                    r = pp.tile([128, 512], f32, name=f"rcp{h}", tag=f"rcp{h}", bufs=2)
                    nc.vector.reciprocal_approx_fast(r[:], allR[:])
                    rcp[h] = r
                for h in pair:
                    nc.vector.tensor_tensor(at[h][:, qs], o_ps[h][:], rcp[h][:], MULT)

            # ---- drive ----
            for c in range(6):
                for _, fn in proj_units(0, c):
                    fn()

            for qb in range(TB):
                if qb + 1 < TB:
                    emit_xt_dma(qb + 1)
                    for c in range(6):
                        fill_proj.extend(proj_units(qb + 1, c))
                phase2_pair(qb, (0, 1))
                phase2_pair(qb, (2, 3))
                # proj chains for tb=qb+1 must land before scores(qb+1)
                while fill_proj:
                    fill_proj.popleft()[1]()
                for ti in range(4 * qb, 4 * qb + 4):
                    fill_oproj.extend(oproj_units(ti))
            while fill_oproj:
                fill_oproj.popleft()[1]()

    nc.compile()
    return nc


def _get_compiled():
    global _compiled
    if _compiled is None:
        _compiled = _build()
    return _compiled


def _host_inputs(x, wq, bq, wkv, bkv, wo):
    import jax.numpy as jnp

    def to_bf16(a):
        return np.asarray(jnp.asarray(a, dtype=jnp.bfloat16))

    pos = np.arange(T, dtype=np.float32)[:, None]
    i = np.arange(0, D, 2, dtype=np.float32)
    inv = np.exp(-(np.log(10000.0) * i / D))
    ang = pos * inv
    pe = np.zeros((T, D), np.float32)
    pe[:, 0::2] = np.sin(ang)
    pe[:, 1::2] = np.cos(ang)
    pet = np.ascontiguousarray(pe.T)                       # [D, T]

    # causal masks for the 4 diagonal tiles of a 512-wide tq block:
    # mask_j[p, c] = 1 if c >= 128*j + p
    c = np.arange(512)[None, :]
    p = np.arange(128)[:, None]
    msk = to_bf16(np.stack([(c >= 128 * j + p) for j in range(4)]).astype(np.float32))
    idb = to_bf16(np.eye(128, dtype=np.float32))

    xts = [to_bf16(np.ascontiguousarray(
        x[b].T.reshape(NE, 128, T).transpose(1, 0, 2))) for b in range(B)]
    in_maps = []
    for core in range(8):
        b, g = divmod(core, G)
        bq_g = bq[g * NQ:(g + 1) * NQ].reshape(QPG, D)     # [h, d]
        wqkv = np.concatenate(
            [wq[:, g * NQ:(g + 1) * NQ], wkv[:, g * NKV:(g + 1) * NKV]], axis=1)
        in_maps.append({
            "xt": xts[b],
            "wqkv": to_bf16(np.ascontiguousarray(
                wqkv.reshape(NE, 128, NQ + NKV).transpose(1, 0, 2))),
            "wo": to_bf16(wo[g * NQ:(g + 1) * NQ, :]),
            "pet": to_bf16(pet),
            "bq": np.ascontiguousarray(bq_g.T).astype(np.float32),
            "bk": np.ascontiguousarray(
                bkv[g * NKV:g * NKV + D].reshape(D, 1)).astype(np.float32),
            "bv": np.ascontiguousarray(
                bkv[g * NKV + D:(g + 1) * NKV].reshape(D, 1)).astype(np.float32),
            "msk": msk,
            "idb": idb,
        })
    return in_maps


def run(x, wq, bq, wkv, bkv, wo, trace=False):
    from concourse.bass_utils import run_bass_kernel_spmd

    nc = _get_compiled()
    in_maps = _host_inputs(
        np.asarray(x, np.float32), np.asarray(wq, np.float32),
        np.asarray(bq, np.float32), np.asarray(wkv, np.float32),
        np.asarray(bkv, np.float32), np.asarray(wo, np.float32),
    )
    res = run_bass_kernel_spmd(nc, in_maps, core_ids=list(range(8)), trace=trace)
    out = np.zeros((B, T, E), np.float32)
    for core in range(8):
        b = core // G
        out[b] += np.asarray(res.results[core]["out"], dtype=np.float32)
    return out, res


def kernel(x, wq, bq, wkv, bkv, wo):
    out, _ = run(x, wq, bq, wkv, bkv, wo, trace=False)
    return out


# revision 12
# speedup vs baseline: 1.0375x; 1.0352x over previous
"""GQA kernel for Trainium2: B=2,T=2048,E=2048,G=4,QPG=4,D=128, causal + sinusoidal PE.

Sharding: one core per (batch, kv-group) pair = 2*4 = 8 cores.
Each core computes q/k/v projections for its group, attention for its 4 query
heads, and a partial output projection (its group's 512 columns of wo);
partials are summed on the host.

v2 layout strategy (per core), all matmul operands bf16:
  - projections: weight tile stationary, x^T streams; per (tb, chain) one
    512-col psum chain over 16 e-tiles. pe^T+bias is folded host-side into
    petq/petk so each drain is a single DVE add.
  - scores S^T[tk, tq]: kt tile stationary, q^T streams 512 cols; exp'd on
    ACT into bf16 P^T tiles (no max subtraction; scores bounded).
  - PV is flipped vs v1: V tile [tk, d] is stationary and P^T streams 512
    cols, producing O^T[d, tq] directly in psum (no output transposes).
  - softmax denominators: R = sum_tk P^T accumulated on DVE, then gpsimd
    partition_all_reduce replicates the column sums to all partitions;
    reciprocal multiplies O^T elementwise during the psum drain.
  - phases are software-pipelined: the scalar engine's exp (~570ns/tile) is
    ~2.6x slower than a score matmul, so projection half-chains of block
    tb+1 and output-projection units of block qb-1 are interleaved into the
    scores stretch one unit per accumulated lag quantum.
"""
import sys

sys.path.insert(0, "/opt/trn_rl_repo")

import math
import numpy as np

B, T, E = 2, 2048, 2048
G, QPG, D = 4, 4, 128
NQ = QPG * D          # 512 q columns per group
NKV = 2 * D           # 256 kv columns per group
TT = T // 128         # 16 t-tiles
TB = T // 512         # 4 t-blocks
NE = E // 128         # 16 e-tiles
ISD = 1.0 / math.sqrt(D)

_compiled = None


def _build():
    from collections import deque
    from concourse import bacc, tile, mybir, bass_isa

    f32 = mybir.dt.float32
    bf16 = mybir.dt.bfloat16
    ADD = mybir.AluOpType.add
    MULT = mybir.AluOpType.mult
    EXP = mybir.ActivationFunctionType.Exp
    IDENT = mybir.ActivationFunctionType.Identity
    RADD = bass_isa.ReduceOp.add

    nc = bacc.Bacc("TRN2", target_bir_lowering=False, debug=False, num_devices=8)

    xt_d = nc.dram_tensor("xt", [128, NE, T], bf16, kind="ExternalInput")    # x^T packed
    wqkv_d = nc.dram_tensor("wqkv", [128, NE, NQ + NKV], bf16, kind="ExternalInput")
    wo_d = nc.dram_tensor("wo", [NQ, E], bf16, kind="ExternalInput")        # group slice
    pet_d = nc.dram_tensor("pet", [D, T], bf16, kind="ExternalInput")         # pe^T
    bq_d = nc.dram_tensor("bq", [D, QPG], f32, kind="ExternalInput")
    bk_d = nc.dram_tensor("bk", [D, 1], f32, kind="ExternalInput")
    bv_d = nc.dram_tensor("bv", [D, 1], f32, kind="ExternalInput")
    msk_d = nc.dram_tensor("msk", [4, 128, 512], bf16, kind="ExternalInput")
    idb_d = nc.dram_tensor("idb", [128, 128], bf16, kind="ExternalInput")
    out_d = nc.dram_tensor("out", [T, E], bf16, kind="ExternalOutput")

    with tile.TileContext(nc) as tc:
        with (
            tc.tile_pool(name="main", bufs=1) as pp,
            tc.tile_pool(name="ps", bufs=1, space="PSUM") as ps,
        ):
            # ---- persistent constants (gpsimd DMA queue; phase-1 stream is
            # on sync so these never delay the first matmuls) ----
            bv = pp.tile([D, 1], f32)
            nc.gpsimd.dma_start(bv[:], bv_d[:])
            bq = pp.tile([D, QPG], f32)
            nc.gpsimd.dma_start(bq[:], bq_d[:])
            bk = pp.tile([D, 1], f32)
            nc.gpsimd.dma_start(bk[:], bk_d[:])
            idb = pp.tile([128, 128], bf16)
            nc.gpsimd.dma_start(idb[:], idb_d[:])
            pet = pp.tile([D, T], bf16)
            msk = [pp.tile([128, 512], bf16, name=f"msk{j}", tag=f"msk{j}") for j in range(4)]
            wo_sb = [pp.tile([128, E], bf16, name=f"wo{h}", tag=f"wo{h}") for h in range(QPG)]

            # ---- persistent activations ----
            qt = [pp.tile([128, T], bf16, name=f"qt{h}", tag=f"qt{h}") for h in range(QPG)]
            kt = pp.tile([128, T], bf16)
            at = [pp.tile([128, T], bf16, name=f"at{h}", tag=f"at{h}") for h in range(QPG)]
            vxall = pp.tile([128, T], bf16, name="vxall", tag="vxall")

            # ---- phase-1 weights + x^T stream (4-e-chunk DMAs, split queues) ----
            wqkv_sb = [pp.tile([128, 4, NQ + NKV], bf16, name=f"wqkv{g}", tag=f"wqkv{g}")
                       for g in range(4)]
            xt_t = [[None] * 4 for _ in range(TB)]

            def emit_xt_dma(tb):
                ts = slice(tb * 512, (tb + 1) * 512)
                for g in range(4):
                    xt_t[tb][g] = pp.tile([128, 4, 512], bf16, name="xt", tag="xt", bufs=6)
                    nc.sync.dma_start(xt_t[tb][g][:], xt_d[:, 4 * g:4 * g + 4, ts])

            # block-0 feed is latency-critical: per-e DMAs spread over four
            # otherwise-idle engine queues so the first chains start immediately
            for g in range(4):
                xt_t[0][g] = pp.tile([128, 4, 512], bf16, name="xt", tag="xt", bufs=6)
            qs3 = [nc.sync, nc.scalar, nc.gpsimd]
            for e in range(NE):
                g, i = divmod(e, 4)
                qs3[(2 * e) % 3].dma_start(wqkv_sb[g][:, i, :], wqkv_d[:, e, :])
                qs3[(2 * e + 1) % 3].dma_start(xt_t[0][g][:, i, :], xt_d[:, e, 0:512])
            nc.scalar.dma_start(pet[:], pet_d[:])
            for j in range(4):
                nc.gpsimd.dma_start(msk[j][:], msk_d[j])
            for h in range(QPG):
                nc.scalar.dma_start(wo_sb[h][:], wo_d[h * 128:(h + 1) * 128, :])

            # ---- unit generators; units are (cost_ns, fn) ----
            def proj_units(tb, c):
                # c: 0-3 = q head c, 4 = k, 5 = v; two half-chain units
                ts = slice(tb * 512, (tb + 1) * 512)
                state = {}

                def wsel(e):
                    g, i = divmod(e, 4)
                    if c < 4:
                        return wqkv_sb[g][:, i, c * 128:(c + 1) * 128]
                    if c == 4:
                        return wqkv_sb[g][:, i, NQ:NQ + 128]
                    return wqkv_sb[g][:, i, NQ + 128:NQ + 256]

                def half_a():
                    ps_t = ps.tile([128, 512], f32, name="big", tag="big", bufs=3)
                    state["ps"] = ps_t
                    for e in range(8):
                        nc.tensor.matmul(ps_t[:], wsel(e), xt_t[tb][e // 4][:, e % 4, :],
                                         start=(e == 0), stop=False)

                def half_b():
                    ps_t = state["ps"]
                    for e in range(8, NE):
                        nc.tensor.matmul(ps_t[:], wsel(e), xt_t[tb][e // 4][:, e % 4, :],
                                         start=False, stop=(e == NE - 1))

                def drain():
                    # emitted one unit late: ps_t is already complete when this
                    # lands in the DVE queue, so masks/exps behind it never block
                    ps_t = state["ps"]
                    if c < 4:
                        nc.vector.tensor_tensor(ps_t[:], ps_t[:],
                                                bq[:, c:c + 1].to_broadcast([128, 512]), ADD)
                        nc.vector.tensor_tensor(qt[c][:, ts], ps_t[:], pet[:, ts], ADD)
                    elif c == 4:
                        nc.vector.tensor_tensor(ps_t[:], ps_t[:],
                                                bk[:].to_broadcast([128, 512]), ADD)
                        nc.vector.tensor_tensor(kt[:, ts], ps_t[:], pet[:, ts], ADD)
                    else:
                        vtb = pp.tile([128, 512], bf16, name="vtb", tag="vtb", bufs=2)
                        nc.vector.tensor_tensor(vtb[:], ps_t[:],
                                                bv[:].to_broadcast([128, 512]), ADD)
                        vtp = ps.tile([128, 512], bf16, name="vtp", tag="vtp", bufs=1)
                        for i in range(4):
                            nc.tensor.transpose(vtp[:, i * 128:(i + 1) * 128],
                                                vtb[:, i * 128:(i + 1) * 128], idb[:])
                        nc.vector.tensor_copy(vxall[:, ts], vtp[:])

                return [(1800, half_a), (1800, half_b), (300, drain)]

            def oproj_units(ti):
                state = {}

                def alloc():
                    state["o"] = pp.tile([128, E], bf16, name="osb", tag="osb", bufs=2)

                def drain_prev():
                    # deferred drain: the stashed w_ps finished on PE during the
                    # previous unit, so this never blocks the DVE/ACT queue
                    eo, w_ps = state["pend"]
                    dst = state["o"][:, eo * 512:(eo + 1) * 512]
                    if eo % 2 == 0:
                        nc.vector.tensor_copy(dst, w_ps[:])
                    else:
                        nc.scalar.activation(dst, w_ps[:], IDENT, scale=1.0)

                units = [(0, alloc)]
                for eo in range(4):
                    def one(eo=eo):
                        if eo > 0:
                            drain_prev()
                        w_ps = ps.tile([128, 512], f32, name="big", tag="big", bufs=3)
                        for h in range(QPG):
                            nc.tensor.matmul(
                                w_ps[:], at[h][:, ti * 128:(ti + 1) * 128],
                                wo_sb[h][:, eo * 512:(eo + 1) * 512],
                                start=(h == 0), stop=(h == QPG - 1),
                            )
                        state["pend"] = (eo, w_ps)
                    units.append((1000, one))

                def store():
                    drain_prev()
                    nc.sync.dma_start(out_d[ti * 128:(ti + 1) * 128, :], state["o"][:])
                units.append((300, store))
                return units

            # two filler queues: proj has a deadline (before next block's
            # scores), oproj is slack-filled
            fill_proj = deque()
            fill_oproj = deque()
            lag = [0]

            def absorb(extra):
                lag[0] += extra
                while lag[0] > 0 and (fill_proj or fill_oproj):
                    q = fill_proj if fill_proj else fill_oproj
                    cost, fn = q.popleft()
                    fn()
                    lag[0] -= cost

            # ---- phase-2 per block qb, heads in pairs. The reciprocal +
            # normalize for a pair are returned as a deferred closure, emitted
            # only after the NEXT pair's scores: the gpsimd allreduce then has
            # a whole scores stretch to complete, so the normalize never
            # head-of-line-blocks the in-order DVE queue ----
            def phase2_pair(qb, pair, pre_pv=None):
                qs = slice(qb * 512, (qb + 1) * 512)
                nkt = 4 * qb + 4
                pts = {h: [] for h in pair}
                R = {h: pp.tile([128, 512], bf16, name=f"R{h}", tag=f"R{h}", bufs=2)
                     for h in pair}
                for tk in range(nkt):
                    for h in pair:
                        s_ps = ps.tile([128, 512], f32, name="s", tag="s", bufs=2)
                        nc.tensor.matmul(s_ps[:], kt[:, tk * 128:(tk + 1) * 128],
                                         qt[h][:, qs], start=True, stop=True)
                        p_t = pp.tile([128, 512], bf16, name="pt", tag="pt", bufs=40)
                        nc.scalar.activation(p_t[:], s_ps[:], EXP, scale=ISD)
                        j = tk - 4 * qb
                        if j >= 0:
                            nc.vector.tensor_tensor(p_t[:], p_t[:], msk[j][:], MULT)
                        pts[h].append(p_t)
                        absorb(460)
                if pre_pv is not None:
                    pre_pv()
                o_ps = {h: ps.tile([128, 512], f32, name="o", tag="o", bufs=2)
                        for h in pair}
                for tk in range(nkt):
                    for h in pair:
                        nc.tensor.matmul(o_ps[h][:], vxall[:, tk * 128:(tk + 1) * 128],
                                         pts[h][tk][:],
                                         start=(tk == 0), stop=(tk == nkt - 1))
                        # R accumulation rides the PV stretch, when DVE is
                        # otherwise idle; it only gates the allreduce below
                        if tk == 0:
                            nc.vector.tensor_copy(R[h][:], pts[h][0][:])
                        else:
                            nc.vector.tensor_tensor(R[h][:], R[h][:], pts[h][tk][:], ADD)
                allR = {}
                for h in pair:
                    a = pp.tile([128, 512], f32, name="allR", tag="allR", bufs=4)
                    nc.gpsimd.partition_all_reduce(a[:], R[h][:], 128, RADD)
                    allR[h] = a

                def finish():
                    for h in pair:
                        r = pp.tile([128, 512], f32, name=f"rcp{h}", tag=f"rcp{h}", bufs=2)
                        nc.vector.reciprocal_approx_fast(r[:], allR[h][:])
                        nc.vector.tensor_tensor(at[h][:, qs], o_ps[h][:], r[:], MULT)
                return finish

            # ---- drive ----
            for c in range(6):
                for _, fn in proj_units(0, c):
                    fn()

            pending = deque()

            def flush_one_pending():
                # previous pair's recip+normalize (+its oproj push, once the
                # whole block's heads are normalized)
                if pending:
                    pending.popleft()()

            for qb in range(TB):
                if qb + 1 < TB:
                    emit_xt_dma(qb + 1)
                    for c in range(6):
                        fill_proj.extend(proj_units(qb + 1, c))
                for pair in ((0, 1), (2, 3)):
                    fin = phase2_pair(qb, pair, pre_pv=flush_one_pending)

                    def fin_and_push(qb=qb, pair=pair, fin=fin):
                        fin()
                        if pair == (2, 3):
                            for ti in range(4 * qb, 4 * qb + 4):
                                fill_oproj.extend(oproj_units(ti))
                    pending.append(fin_and_push)
                # proj chains for tb=qb+1 must land before scores(qb+1)
                while fill_proj:
                    fill_proj.popleft()[1]()
            while pending:
                pending.popleft()()
            while fill_oproj:
                fill_oproj.popleft()[1]()

    nc.compile()
    return nc


def _get_compiled():
    global _compiled
    if _compiled is None:
        _compiled = _build()
    return _compiled


def _host_inputs(x, wq, bq, wkv, bkv, wo):
    import jax.numpy as jnp

    def to_bf16(a):
        return np.asarray(jnp.asarray(a, dtype=jnp.bfloat16))

    pos = np.arange(T, dtype=np.float32)[:, None]
    i = np.arange(0, D, 2, dtype=np.float32)
    inv = np.exp(-(np.log(10000.0) * i / D))
    ang = pos * inv
    pe = np.zeros((T, D), np.float32)
    pe[:, 0::2] = np.sin(ang)
    pe[:, 1::2] = np.cos(ang)
    pet = np.ascontiguousarray(pe.T)                       # [D, T]

    # causal masks for the 4 diagonal tiles of a 512-wide tq block:
    # mask_j[p, c] = 1 if c >= 128*j + p
    c = np.arange(512)[None, :]
    p = np.arange(128)[:, None]
    msk = to_bf16(np.stack([(c >= 128 * j + p) for j in range(4)]).astype(np.float32))
    idb = to_bf16(np.eye(128, dtype=np.float32))

    xts = [to_bf16(np.ascontiguousarray(
        x[b].T.reshape(NE, 128, T).transpose(1, 0, 2))) for b in range(B)]
    in_maps = []
    for core in range(8):
        b, g = divmod(core, G)
        bq_g = bq[g * NQ:(g + 1) * NQ].reshape(QPG, D)     # [h, d]
        wqkv = np.concatenate(
            [wq[:, g * NQ:(g + 1) * NQ], wkv[:, g * NKV:(g + 1) * NKV]], axis=1)
        in_maps.append({
            "xt": xts[b],
            "wqkv": to_bf16(np.ascontiguousarray(
                wqkv.reshape(NE, 128, NQ + NKV).transpose(1, 0, 2))),
            "wo": to_bf16(wo[g * NQ:(g + 1) * NQ, :]),
            "pet": to_bf16(pet),
            "bq": np.ascontiguousarray(bq_g.T).astype(np.float32),
            "bk": np.ascontiguousarray(
                bkv[g * NKV:g * NKV + D].reshape(D, 1)).astype(np.float32),
            "bv": np.ascontiguousarray(
                bkv[g * NKV + D:(g + 1) * NKV].reshape(D, 1)).astype(np.float32),
            "msk": msk,
            "idb": idb,
        })
    return in_maps


def run(x, wq, bq, wkv, bkv, wo, trace=False):
    from concourse.bass_utils import run_bass_kernel_spmd

    nc = _get_compiled()
    in_maps = _host_inputs(
        np.asarray(x, np.float32), np.asarray(wq, np.float32),
        np.asarray(bq, np.float32), np.asarray(wkv, np.float32),
        np.asarray(bkv, np.float32), np.asarray(wo, np.float32),
    )
    res = run_bass_kernel_spmd(nc, in_maps, core_ids=list(range(8)), trace=trace)
    out = np.zeros((B, T, E), np.float32)
    for core in range(8):
        b = core // G
        out[b] += np.asarray(res.results[core]["out"], dtype=np.float32)
    return out, res


def kernel(x, wq, bq, wkv, bkv, wo):
    out, _ = run(x, wq, bq, wkv, bkv, wo, trace=False)
    return out


# revision 13
# speedup vs baseline: 1.0732x; 1.0344x over previous
"""GQA kernel for Trainium2: B=2,T=2048,E=2048,G=4,QPG=4,D=128, causal + sinusoidal PE.

Sharding: one core per (batch, kv-group) pair = 2*4 = 8 cores.
Each core computes q/k/v projections for its group, attention for its 4 query
heads, and a partial output projection (its group's 512 columns of wo);
partials are summed on the host.

v2 layout strategy (per core), all matmul operands bf16:
  - projections: weight tile stationary, x^T streams; per (tb, chain) one
    512-col psum chain over 16 e-tiles. pe^T+bias is folded host-side into
    petq/petk so each drain is a single DVE add.
  - scores S^T[tk, tq]: kt tile stationary, q^T streams 512 cols; exp'd on
    ACT into bf16 P^T tiles (no max subtraction; scores bounded).
  - PV is flipped vs v1: V tile [tk, d] is stationary and P^T streams 512
    cols, producing O^T[d, tq] directly in psum (no output transposes).
  - softmax denominators: R = sum_tk P^T accumulated on DVE, then gpsimd
    partition_all_reduce replicates the column sums to all partitions;
    reciprocal multiplies O^T elementwise during the psum drain.
  - phases are software-pipelined: the scalar engine's exp (~570ns/tile) is
    ~2.6x slower than a score matmul, so projection half-chains of block
    tb+1 and output-projection units of block qb-1 are interleaved into the
    scores stretch one unit per accumulated lag quantum.
"""
import sys

sys.path.insert(0, "/opt/trn_rl_repo")

import math
import numpy as np

B, T, E = 2, 2048, 2048
G, QPG, D = 4, 4, 128
NQ = QPG * D          # 512 q columns per group
NKV = 2 * D           # 256 kv columns per group
TT = T // 128         # 16 t-tiles
TB = T // 512         # 4 t-blocks
NE = E // 128         # 16 e-tiles
ISD = 1.0 / math.sqrt(D)

_compiled = None


def _build():
    from collections import deque
    from concourse import bacc, tile, mybir, bass_isa

    f32 = mybir.dt.float32
    bf16 = mybir.dt.bfloat16
    ADD = mybir.AluOpType.add
    MULT = mybir.AluOpType.mult
    EXP = mybir.ActivationFunctionType.Exp
    IDENT = mybir.ActivationFunctionType.Identity
    RADD = bass_isa.ReduceOp.add

    nc = bacc.Bacc("TRN2", target_bir_lowering=False, debug=False, num_devices=8)

    xt_d = nc.dram_tensor("xt", [128, NE, T], bf16, kind="ExternalInput")    # x^T packed
    wqkv_d = nc.dram_tensor("wqkv", [128, NE, NQ + NKV], bf16, kind="ExternalInput")
    wo_d = nc.dram_tensor("wo", [NQ, E], bf16, kind="ExternalInput")        # group slice
    pet_d = nc.dram_tensor("pet", [D, T], bf16, kind="ExternalInput")         # pe^T
    bq_d = nc.dram_tensor("bq", [D, QPG], f32, kind="ExternalInput")
    bk_d = nc.dram_tensor("bk", [D, 1], f32, kind="ExternalInput")
    bv_d = nc.dram_tensor("bv", [D, 1], f32, kind="ExternalInput")
    msk_d = nc.dram_tensor("msk", [4, 128, 512], bf16, kind="ExternalInput")
    idb_d = nc.dram_tensor("idb", [128, 128], bf16, kind="ExternalInput")
    out_d = nc.dram_tensor("out", [T, E], bf16, kind="ExternalOutput")

    with tile.TileContext(nc) as tc:
        with (
            tc.tile_pool(name="main", bufs=1) as pp,
            tc.tile_pool(name="ps", bufs=1, space="PSUM") as ps,
        ):
            # ---- persistent constants (gpsimd DMA queue; phase-1 stream is
            # on sync so these never delay the first matmuls) ----
            bv = pp.tile([D, 1], f32)
            nc.gpsimd.dma_start(bv[:], bv_d[:])
            bq = pp.tile([D, QPG], f32)
            nc.gpsimd.dma_start(bq[:], bq_d[:])
            bk = pp.tile([D, 1], f32)
            nc.gpsimd.dma_start(bk[:], bk_d[:])
            idb = pp.tile([128, 128], bf16)
            nc.gpsimd.dma_start(idb[:], idb_d[:])
            pet = pp.tile([D, T], bf16)
            msk = [pp.tile([128, 512], bf16, name=f"msk{j}", tag=f"msk{j}") for j in range(4)]
            wo_sb = [pp.tile([128, E], bf16, name=f"wo{h}", tag=f"wo{h}") for h in range(QPG)]

            # ---- persistent activations ----
            qt = [pp.tile([128, T], bf16, name=f"qt{h}", tag=f"qt{h}") for h in range(QPG)]
            kt = pp.tile([128, T], bf16)
            at = [pp.tile([128, T], bf16, name=f"at{h}", tag=f"at{h}") for h in range(QPG)]
            vxall = pp.tile([128, T], bf16, name="vxall", tag="vxall")

            # ---- phase-1 weights + x^T stream (4-e-chunk DMAs, split queues) ----
            wqkv_sb = [pp.tile([128, 4, NQ + NKV], bf16, name=f"wqkv{g}", tag=f"wqkv{g}")
                       for g in range(4)]
            xt_t = [[None] * 4 for _ in range(TB)]

            def emit_xt_dma(tb):
                ts = slice(tb * 512, (tb + 1) * 512)
                for g in range(4):
                    xt_t[tb][g] = pp.tile([128, 4, 512], bf16, name="xt", tag="xt", bufs=6)
                    nc.sync.dma_start(xt_t[tb][g][:], xt_d[:, 4 * g:4 * g + 4, ts])

            # block-0 feed is latency-critical: per-e DMAs spread over four
            # otherwise-idle engine queues so the first chains start immediately
            for g in range(4):
                xt_t[0][g] = pp.tile([128, 4, 512], bf16, name="xt", tag="xt", bufs=6)
            qs3 = [nc.sync, nc.scalar, nc.gpsimd]
            for e in range(NE):
                g, i = divmod(e, 4)
                qs3[(2 * e) % 3].dma_start(wqkv_sb[g][:, i, :], wqkv_d[:, e, :])
                qs3[(2 * e + 1) % 3].dma_start(xt_t[0][g][:, i, :], xt_d[:, e, 0:512])
            nc.scalar.dma_start(pet[:], pet_d[:])
            for j in range(4):
                nc.gpsimd.dma_start(msk[j][:], msk_d[j])
            for h in range(QPG):
                nc.scalar.dma_start(wo_sb[h][:], wo_d[h * 128:(h + 1) * 128, :])

            # ---- unit generators; units are (cost_ns, fn) ----
            def proj_units(tb, c):
                # c: 0-3 = q head c, 4 = k, 5 = v; two half-chain units
                ts = slice(tb * 512, (tb + 1) * 512)
                state = {}

                def wsel(e):
                    g, i = divmod(e, 4)
                    if c < 4:
                        return wqkv_sb[g][:, i, c * 128:(c + 1) * 128]
                    if c == 4:
                        return wqkv_sb[g][:, i, NQ:NQ + 128]
                    return wqkv_sb[g][:, i, NQ + 128:NQ + 256]

                def half_a():
                    ps_t = ps.tile([128, 512], f32, name="big", tag="big", bufs=3)
                    state["ps"] = ps_t
                    for e in range(8):
                        nc.tensor.matmul(ps_t[:], wsel(e), xt_t[tb][e // 4][:, e % 4, :],
                                         start=(e == 0), stop=False)

                def half_b():
                    ps_t = state["ps"]
                    for e in range(8, NE):
                        nc.tensor.matmul(ps_t[:], wsel(e), xt_t[tb][e // 4][:, e % 4, :],
                                         start=False, stop=(e == NE - 1))

                def drain():
                    # emitted one unit late: ps_t is already complete when this
                    # lands in the DVE queue, so masks/exps behind it never block
                    ps_t = state["ps"]
                    if c < 4:
                        nc.vector.tensor_tensor(ps_t[:], ps_t[:],
                                                bq[:, c:c + 1].to_broadcast([128, 512]), ADD)
                        nc.vector.tensor_tensor(qt[c][:, ts], ps_t[:], pet[:, ts], ADD)
                    elif c == 4:
                        nc.vector.tensor_tensor(ps_t[:], ps_t[:],
                                                bk[:].to_broadcast([128, 512]), ADD)
                        nc.vector.tensor_tensor(kt[:, ts], ps_t[:], pet[:, ts], ADD)
                    else:
                        vtb = pp.tile([128, 512], bf16, name="vtb", tag="vtb", bufs=2)
                        nc.vector.tensor_tensor(vtb[:], ps_t[:],
                                                bv[:].to_broadcast([128, 512]), ADD)
                        vtp = ps.tile([128, 512], bf16, name="vtp", tag="vtp", bufs=1)
                        for i in range(4):
                            nc.tensor.transpose(vtp[:, i * 128:(i + 1) * 128],
                                                vtb[:, i * 128:(i + 1) * 128], idb[:])
                        nc.vector.tensor_copy(vxall[:, ts], vtp[:])

                return [(1800, half_a), (1800, half_b), (300, drain)]

            def oproj_units(ti):
                state = {}

                def alloc():
                    state["o"] = pp.tile([128, E], bf16, name="osb", tag="osb", bufs=2)

                def drain_prev():
                    # deferred drain: the stashed w_ps finished on PE during the
                    # previous unit, so this never blocks the DVE/ACT queue
                    eo, w_ps = state["pend"]
                    dst = state["o"][:, eo * 512:(eo + 1) * 512]
                    if eo % 2 == 0:
                        nc.vector.tensor_copy(dst, w_ps[:])
                    else:
                        nc.scalar.activation(dst, w_ps[:], IDENT, scale=1.0)

                units = [(0, alloc)]
                for eo in range(4):
                    def one(eo=eo):
                        if eo > 0:
                            drain_prev()
                        w_ps = ps.tile([128, 512], f32, name="big", tag="big", bufs=3)
                        for h in range(QPG):
                            nc.tensor.matmul(
                                w_ps[:], at[h][:, ti * 128:(ti + 1) * 128],
                                wo_sb[h][:, eo * 512:(eo + 1) * 512],
                                start=(h == 0), stop=(h == QPG - 1),
                            )
                        state["pend"] = (eo, w_ps)
                    units.append((1000, one))

                def store():
                    drain_prev()
                    nc.sync.dma_start(out_d[ti * 128:(ti + 1) * 128, :], state["o"][:])
                units.append((300, store))
                return units

            # two filler queues: proj has a deadline (before next block's
            # scores), oproj is slack-filled
            fill_proj = deque()
            fill_oproj = deque()
            lag = [0]

            def absorb(extra):
                lag[0] += extra
                while lag[0] > 0 and (fill_proj or fill_oproj):
                    q = fill_proj if fill_proj else fill_oproj
                    cost, fn = q.popleft()
                    fn()
                    lag[0] -= cost

            # ---- phase-2 per block qb, heads in pairs. The reciprocal +
            # normalize for a pair are returned as a deferred closure, emitted
            # only after the NEXT pair's scores: the gpsimd allreduce then has
            # a whole scores stretch to complete, so the normalize never
            # head-of-line-blocks the in-order DVE queue ----
            def phase2_pair(qb, pair, pre_pv=None):
                qs = slice(qb * 512, (qb + 1) * 512)
                nkt = 4 * qb + 4
                pts = {h: [] for h in pair}
                R = {h: pp.tile([128, 512], bf16, name=f"R{h}", tag=f"R{h}", bufs=2)
                     for h in pair}
                for tk in range(nkt):
                    for h in pair:
                        s_ps = ps.tile([128, 512], f32, name="s", tag="s", bufs=2)
                        nc.tensor.matmul(s_ps[:], kt[:, tk * 128:(tk + 1) * 128],
                                         qt[h][:, qs], start=True, stop=True)
                        p_t = pp.tile([128, 512], bf16, name="pt", tag="pt", bufs=40)
                        nc.scalar.activation(p_t[:], s_ps[:], EXP, scale=ISD)
                        j = tk - 4 * qb
                        if j >= 0:
                            nc.vector.tensor_tensor(p_t[:], p_t[:], msk[j][:], MULT)
                        pts[h].append(p_t)
                        absorb(460)
                if pre_pv is not None:
                    pre_pv()
                o_ps = {h: ps.tile([128, 512], f32, name="o", tag="o", bufs=2)
                        for h in pair}
                for tk in range(nkt):
                    for h in pair:
                        nc.tensor.matmul(o_ps[h][:], vxall[:, tk * 128:(tk + 1) * 128],
                                         pts[h][tk][:],
                                         start=(tk == 0), stop=(tk == nkt - 1))
                        # R accumulation rides the PV stretch, when DVE is
                        # otherwise idle; it only gates the allreduce below
                        if tk == 0:
                            nc.vector.tensor_copy(R[h][:], pts[h][0][:])
                        else:
                            nc.vector.tensor_tensor(R[h][:], R[h][:], pts[h][tk][:], ADD)
                allR = {}
                for h in pair:
                    a = pp.tile([128, 512], f32, name="allR", tag="allR", bufs=4)
                    nc.gpsimd.partition_all_reduce(a[:], R[h][:], 128, RADD)
                    allR[h] = a

                def finish():
                    for h in pair:
                        r = pp.tile([128, 512], f32, name=f"rcp{h}", tag=f"rcp{h}", bufs=2)
                        nc.vector.reciprocal_approx_fast(r[:], allR[h][:])
                        nc.vector.tensor_tensor(at[h][:, qs], o_ps[h][:], r[:], MULT)
                return finish

            # ---- drive ----
            for c in range(6):
                for _, fn in proj_units(0, c):
                    fn()

            pending = deque()

            def flush_one_pending():
                # previous pair's recip+normalize (+its oproj push, once the
                # whole block's heads are normalized)
                if pending:
                    pending.popleft()()

            for qb in range(TB):
                if qb + 1 < TB:
                    emit_xt_dma(qb + 1)
                    for c in range(6):
                        fill_proj.extend(proj_units(qb + 1, c))
                for pair in ((0, 1), (2, 3)):
                    fin = phase2_pair(qb, pair, pre_pv=flush_one_pending)

                    def fin_and_push(qb=qb, pair=pair, fin=fin):
                        fin()
                        if pair == (2, 3):
                            for ti in range(4 * qb, 4 * qb + 4):
                                fill_oproj.extend(oproj_units(ti))
                    pending.append(fin_and_push)
                # proj chains for tb=qb+1 must land before scores(qb+1); they
                # also cover the final pair's allreduce latency, topped up with
                # a couple of backlog oproj units, so the block's last finish
                # never stalls the DVE queue
                while fill_proj:
                    fill_proj.popleft()[1]()
                for _ in range(2):
                    if fill_oproj:
                        fill_oproj.popleft()[1]()
                while pending:
                    pending.popleft()()
            while fill_oproj:
                fill_oproj.popleft()[1]()

    nc.compile()
    return nc


def _get_compiled():
    global _compiled
    if _compiled is None:
        _compiled = _build()
    return _compiled


def _host_inputs(x, wq, bq, wkv, bkv, wo):
    import jax.numpy as jnp

    def to_bf16(a):
        return np.asarray(jnp.asarray(a, dtype=jnp.bfloat16))

    pos = np.arange(T, dtype=np.float32)[:, None]
    i = np.arange(0, D, 2, dtype=np.float32)
    inv = np.exp(-(np.log(10000.0) * i / D))
    ang = pos * inv
    pe = np.zeros((T, D), np.float32)
    pe[:, 0::2] = np.sin(ang)
    pe[:, 1::2] = np.cos(ang)
    pet = np.ascontiguousarray(pe.T)                       # [D, T]

    # causal masks for the 4 diagonal tiles of a 512-wide tq block:
    # mask_j[p, c] = 1 if c >= 128*j + p
    c = np.arange(512)[None, :]
    p = np.arange(128)[:, None]
    msk = to_bf16(np.stack([(c >= 128 * j + p) for j in range(4)]).astype(np.float32))
    idb = to_bf16(np.eye(128, dtype=np.float32))

    xts = [to_bf16(np.ascontiguousarray(
        x[b].T.reshape(NE, 128, T).transpose(1, 0, 2))) for b in range(B)]
    in_maps = []
    for core in range(8):
        b, g = divmod(core, G)
        bq_g = bq[g * NQ:(g + 1) * NQ].reshape(QPG, D)     # [h, d]
        wqkv = np.concatenate(
            [wq[:, g * NQ:(g + 1) * NQ], wkv[:, g * NKV:(g + 1) * NKV]], axis=1)
        in_maps.append({
            "xt": xts[b],
            "wqkv": to_bf16(np.ascontiguousarray(
                wqkv.reshape(NE, 128, NQ + NKV).transpose(1, 0, 2))),
            "wo": to_bf16(wo[g * NQ:(g + 1) * NQ, :]),
            "pet": to_bf16(pet),
            "bq": np.ascontiguousarray(bq_g.T).astype(np.float32),
            "bk": np.ascontiguousarray(
                bkv[g * NKV:g * NKV + D].reshape(D, 1)).astype(np.float32),
            "bv": np.ascontiguousarray(
                bkv[g * NKV + D:(g + 1) * NKV].reshape(D, 1)).astype(np.float32),
            "msk": msk,
            "idb": idb,
        })
    return in_maps


def run(x, wq, bq, wkv, bkv, wo, trace=False):
    from concourse.bass_utils import run_bass_kernel_spmd

    nc = _get_compiled()
    in_maps = _host_inputs(
        np.asarray(x, np.float32), np.asarray(wq, np.float32),
        np.asarray(bq, np.float32), np.asarray(wkv, np.float32),
        np.asarray(bkv, np.float32), np.asarray(wo, np.float32),
    )
    res = run_bass_kernel_spmd(nc, in_maps, core_ids=list(range(8)), trace=trace)
    out = np.zeros((B, T, E), np.float32)
    for core in range(8):
        b = core // G
        out[b] += np.asarray(res.results[core]["out"], dtype=np.float32)
    return out, res


def kernel(x, wq, bq, wkv, bkv, wo):
    out, _ = run(x, wq, bq, wkv, bkv, wo, trace=False)
    return out


# revision 15
# speedup vs baseline: 1.0749x; 1.0016x over previous
"""GQA kernel for Trainium2: B=2,T=2048,E=2048,G=4,QPG=4,D=128, causal + sinusoidal PE.

Sharding: one core per (batch, kv-group) pair = 2*4 = 8 cores.
Each core computes q/k/v projections for its group, attention for its 4 query
heads, and a partial output projection (its group's 512 columns of wo);
partials are summed on the host.

v2 layout strategy (per core), all matmul operands bf16:
  - projections: weight tile stationary, x^T streams; per (tb, chain) one
    512-col psum chain over 16 e-tiles. pe^T+bias is folded host-side into
    petq/petk so each drain is a single DVE add.
  - scores S^T[tk, tq]: kt tile stationary, q^T streams 512 cols; exp'd on
    ACT into bf16 P^T tiles (no max subtraction; scores bounded).
  - PV is flipped vs v1: V tile [tk, d] is stationary and P^T streams 512
    cols, producing O^T[d, tq] directly in psum (no output transposes).
  - softmax denominators: R = sum_tk P^T accumulated on DVE, then gpsimd
    partition_all_reduce replicates the column sums to all partitions;
    reciprocal multiplies O^T elementwise during the psum drain.
  - phases are software-pipelined: the scalar engine's exp (~570ns/tile) is
    ~2.6x slower than a score matmul, so projection half-chains of block
    tb+1 and output-projection units of block qb-1 are interleaved into the
    scores stretch one unit per accumulated lag quantum.
"""
import sys

sys.path.insert(0, "/opt/trn_rl_repo")

import math
import numpy as np

B, T, E = 2, 2048, 2048
G, QPG, D = 4, 4, 128
NQ = QPG * D          # 512 q columns per group
NKV = 2 * D           # 256 kv columns per group
TT = T // 128         # 16 t-tiles
TB = T // 512         # 4 t-blocks
NE = E // 128         # 16 e-tiles
ISD = 1.0 / math.sqrt(D)

_compiled = None


def _build():
    from collections import deque
    from concourse import bacc, tile, mybir, bass_isa

    f32 = mybir.dt.float32
    bf16 = mybir.dt.bfloat16
    ADD = mybir.AluOpType.add
    MULT = mybir.AluOpType.mult
    EXP = mybir.ActivationFunctionType.Exp
    IDENT = mybir.ActivationFunctionType.Identity
    RADD = bass_isa.ReduceOp.add

    nc = bacc.Bacc("TRN2", target_bir_lowering=False, debug=False, num_devices=8)

    xt_d = nc.dram_tensor("xt", [128, NE, T], bf16, kind="ExternalInput")    # x^T packed
    wqkv_d = nc.dram_tensor("wqkv", [128, NE, NQ + NKV], bf16, kind="ExternalInput")
    wo_d = nc.dram_tensor("wo", [NQ, E], bf16, kind="ExternalInput")        # group slice
    pet_d = nc.dram_tensor("pet", [D, T], bf16, kind="ExternalInput")         # pe^T
    bq_d = nc.dram_tensor("bq", [D, QPG], f32, kind="ExternalInput")
    bk_d = nc.dram_tensor("bk", [D, 1], f32, kind="ExternalInput")
    bv_d = nc.dram_tensor("bv", [D, 1], f32, kind="ExternalInput")
    msk_d = nc.dram_tensor("msk", [4, 128, 512], bf16, kind="ExternalInput")
    idb_d = nc.dram_tensor("idb", [128, 128], bf16, kind="ExternalInput")
    out_d = nc.dram_tensor("out", [T, E], bf16, kind="ExternalOutput")

    with tile.TileContext(nc) as tc:
        with (
            tc.tile_pool(name="main", bufs=1) as pp,
            tc.tile_pool(name="ps", bufs=1, space="PSUM") as ps,
        ):
            # ---- persistent constants (gpsimd DMA queue; phase-1 stream is
            # on sync so these never delay the first matmuls) ----
            bv = pp.tile([D, 1], f32)
            nc.gpsimd.dma_start(bv[:], bv_d[:])
            bq = pp.tile([D, QPG], f32)
            nc.gpsimd.dma_start(bq[:], bq_d[:])
            bk = pp.tile([D, 1], f32)
            nc.gpsimd.dma_start(bk[:], bk_d[:])
            idb = pp.tile([128, 128], bf16)
            nc.gpsimd.dma_start(idb[:], idb_d[:])
            pet = pp.tile([D, T], bf16)
            msk = [pp.tile([128, 512], bf16, name=f"msk{j}", tag=f"msk{j}") for j in range(4)]
            wo_sb = [pp.tile([128, E], bf16, name=f"wo{h}", tag=f"wo{h}") for h in range(QPG)]

            # ---- persistent activations ----
            qt = [pp.tile([128, T], bf16, name=f"qt{h}", tag=f"qt{h}") for h in range(QPG)]
            kt = pp.tile([128, T], bf16)
            at = [pp.tile([128, T], bf16, name=f"at{h}", tag=f"at{h}") for h in range(QPG)]
            vxall = pp.tile([128, T], bf16, name="vxall", tag="vxall")

            # ---- phase-1 weights + x^T stream (4-e-chunk DMAs, split queues) ----
            wqkv_sb = [pp.tile([128, 4, NQ + NKV], bf16, name=f"wqkv{g}", tag=f"wqkv{g}")
                       for g in range(4)]
            xt_t = [[None] * 4 for _ in range(TB)]

            def emit_xt_dma(tb):
                ts = slice(tb * 512, (tb + 1) * 512)
                for g in range(4):
                    xt_t[tb][g] = pp.tile([128, 4, 512], bf16, name="xt", tag="xt", bufs=6)
                    nc.sync.dma_start(xt_t[tb][g][:], xt_d[:, 4 * g:4 * g + 4, ts])

            # block-0 feed is latency-critical: per-e DMAs spread over four
            # otherwise-idle engine queues so the first chains start immediately
            for g in range(4):
                xt_t[0][g] = pp.tile([128, 4, 512], bf16, name="xt", tag="xt", bufs=6)
            qs3 = [nc.sync, nc.scalar, nc.gpsimd]
            for e in range(NE):
                g, i = divmod(e, 4)
                qs3[(2 * e) % 3].dma_start(wqkv_sb[g][:, i, :], wqkv_d[:, e, :])
                qs3[(2 * e + 1) % 3].dma_start(xt_t[0][g][:, i, :], xt_d[:, e, 0:512])
            nc.scalar.dma_start(pet[:], pet_d[:])
            for j in range(4):
                nc.gpsimd.dma_start(msk[j][:], msk_d[j])
            for h in range(QPG):
                nc.scalar.dma_start(wo_sb[h][:], wo_d[h * 128:(h + 1) * 128, :])

            # ---- unit generators; units are (cost_ns, fn) ----
            def proj_units(tb, c):
                # c: 0-3 = q head c, 4 = k, 5 = v; two half-chain units
                ts = slice(tb * 512, (tb + 1) * 512)
                state = {}

                def wsel(e):
                    g, i = divmod(e, 4)
                    if c < 4:
                        return wqkv_sb[g][:, i, c * 128:(c + 1) * 128]
                    if c == 4:
                        return wqkv_sb[g][:, i, NQ:NQ + 128]
                    return wqkv_sb[g][:, i, NQ + 128:NQ + 256]

                def half_a():
                    ps_t = ps.tile([128, 512], f32, name="big", tag="big", bufs=3)
                    state["ps"] = ps_t
                    for e in range(8):
                        nc.tensor.matmul(ps_t[:], wsel(e), xt_t[tb][e // 4][:, e % 4, :],
                                         start=(e == 0), stop=False)

                def half_b():
                    ps_t = state["ps"]
                    for e in range(8, NE):
                        nc.tensor.matmul(ps_t[:], wsel(e), xt_t[tb][e // 4][:, e % 4, :],
                                         start=False, stop=(e == NE - 1))

                def drain():
                    # emitted one unit late: ps_t is already complete when this
                    # lands in the DVE queue, so masks/exps behind it never block
                    ps_t = state["ps"]
                    if c < 4:
                        nc.vector.tensor_tensor(ps_t[:], ps_t[:],
                                                bq[:, c:c + 1].to_broadcast([128, 512]), ADD)
                        nc.vector.tensor_tensor(qt[c][:, ts], ps_t[:], pet[:, ts], ADD)
                    elif c == 4:
                        nc.vector.tensor_tensor(ps_t[:], ps_t[:],
                                                bk[:].to_broadcast([128, 512]), ADD)
                        nc.vector.tensor_tensor(kt[:, ts], ps_t[:], pet[:, ts], ADD)
                    else:
                        vtb = pp.tile([128, 512], bf16, name="vtb", tag="vtb", bufs=2)
                        nc.vector.tensor_tensor(vtb[:], ps_t[:],
                                                bv[:].to_broadcast([128, 512]), ADD)
                        vtp = ps.tile([128, 512], bf16, name="vtp", tag="vtp", bufs=1)
                        for i in range(4):
                            nc.tensor.transpose(vtp[:, i * 128:(i + 1) * 128],
                                                vtb[:, i * 128:(i + 1) * 128], idb[:])
                        nc.vector.tensor_copy(vxall[:, ts], vtp[:])

                return [(1800, half_a), (1800, half_b), (300, drain)]

            def oproj_units(ti, last=False):
                state = {}

                def alloc():
                    state["o"] = pp.tile([128, E], bf16, name="osb", tag="osb", bufs=2)

                state["pend"] = []

                def drain_oldest():
                    # drains run two units late and always on DVE: by emission
                    # time the psum is long finished, so neither the exp stream
                    # (ACT) nor the mask stream (DVE) ever waits behind it.
                    # The final block's units run with nothing behind them, so
                    # their drains are immediate (last=True) to shorten the tail
                    eo, w_ps = state["pend"].pop(0)
                    nc.vector.tensor_copy(state["o"][:, eo * 512:(eo + 1) * 512], w_ps[:])

                units = [(0, alloc)]
                for eo in range(4):
                    def one(eo=eo):
                        if len(state["pend"]) >= (1 if last else 2):
                            drain_oldest()
                        w_ps = ps.tile([128, 512], f32, name="big", tag="big", bufs=3)
                        for h in range(QPG):
                            nc.tensor.matmul(
                                w_ps[:], at[h][:, ti * 128:(ti + 1) * 128],
                                wo_sb[h][:, eo * 512:(eo + 1) * 512],
                                start=(h == 0), stop=(h == QPG - 1),
                            )
                        state["pend"].append((eo, w_ps))
                    units.append((1000, one))

                def store():
                    while state["pend"]:
                        drain_oldest()
                    # final stores alternate queues so the transfers overlap
                    eng = nc.scalar if (last and ti % 2) else nc.sync
                    eng.dma_start(out_d[ti * 128:(ti + 1) * 128, :], state["o"][:])
                units.append((300, store))
                return units

            # two filler queues: proj has a deadline (before next block's
            # scores), oproj is slack-filled
            fill_proj = deque()
            fill_oproj = deque()
            lag = [0]

            def absorb(extra):
                lag[0] += extra
                while lag[0] > 0 and (fill_proj or fill_oproj):
                    q = fill_proj if fill_proj else fill_oproj
                    cost, fn = q.popleft()
                    fn()
                    lag[0] -= cost

            # ---- phase-2 per block qb, heads in pairs. The reciprocal +
            # normalize for a pair are returned as a deferred closure, emitted
            # only after the NEXT pair's scores: the gpsimd allreduce then has
            # a whole scores stretch to complete, so the normalize never
            # head-of-line-blocks the in-order DVE queue ----
            def phase2_pair(qb, pair, pre_pv=None):
                qs = slice(qb * 512, (qb + 1) * 512)
                nkt = 4 * qb + 4
                pts = {h: [] for h in pair}
                R = {h: pp.tile([128, 512], bf16, name=f"R{h}", tag=f"R{h}", bufs=2)
                     for h in pair}
                for tk in range(nkt):
                    for h in pair:
                        s_ps = ps.tile([128, 512], f32, name="s", tag="s", bufs=2)
                        nc.tensor.matmul(s_ps[:], kt[:, tk * 128:(tk + 1) * 128],
                                         qt[h][:, qs], start=True, stop=True)
                        p_t = pp.tile([128, 512], bf16, name="pt", tag="pt", bufs=40)
                        nc.scalar.activation(p_t[:], s_ps[:], EXP, scale=ISD)
                        j = tk - 4 * qb
                        if j >= 0:
                            nc.vector.tensor_tensor(p_t[:], p_t[:], msk[j][:], MULT)
                        pts[h].append(p_t)
                        absorb(460)
                if pre_pv is not None:
                    pre_pv()
                o_ps = {h: ps.tile([128, 512], f32, name="o", tag="o", bufs=2)
                        for h in pair}
                for tk in range(nkt):
                    for h in pair:
                        nc.tensor.matmul(o_ps[h][:], vxall[:, tk * 128:(tk + 1) * 128],
                                         pts[h][tk][:],
                                         start=(tk == 0), stop=(tk == nkt - 1))
                        # R accumulation rides the PV stretch, when DVE is
                        # otherwise idle; it only gates the allreduce below
                        if tk == 0:
                            nc.vector.tensor_copy(R[h][:], pts[h][0][:])
                        else:
                            nc.vector.tensor_tensor(R[h][:], R[h][:], pts[h][tk][:], ADD)
                allR = {}
                for h in pair:
                    a = pp.tile([128, 512], f32, name="allR", tag="allR", bufs=4)
                    nc.gpsimd.partition_all_reduce(a[:], R[h][:], 128, RADD)
                    allR[h] = a

                def finish():
                    for h in pair:
                        r = pp.tile([128, 512], f32, name=f"rcp{h}", tag=f"rcp{h}", bufs=2)
                        nc.vector.reciprocal_approx_fast(r[:], allR[h][:])
                        nc.vector.tensor_tensor(at[h][:, qs], o_ps[h][:], r[:], MULT)
                return finish

            # ---- drive ----
            for c in range(6):
                for _, fn in proj_units(0, c):
                    fn()

            pending = deque()

            def flush_one_pending():
                # previous pair's recip+normalize (+its oproj push, once the
                # whole block's heads are normalized)
                if pending:
                    pending.popleft()()

            for qb in range(TB):
                if qb + 1 < TB:
                    emit_xt_dma(qb + 1)
                    for c in range(6):
                        fill_proj.extend(proj_units(qb + 1, c))
                for pair in ((0, 1), (2, 3)):
                    fin = phase2_pair(qb, pair, pre_pv=flush_one_pending)

                    def fin_and_push(qb=qb, pair=pair, fin=fin):
                        fin()
                        if pair == (2, 3):
                            for ti in range(4 * qb, 4 * qb + 4):
                                fill_oproj.extend(oproj_units(ti, last=(qb == TB - 1)))
                    pending.append(fin_and_push)
                # proj chains for tb=qb+1 must land before scores(qb+1); they
                # also cover the final pair's allreduce latency, topped up with
                # a couple of backlog oproj units, so the block's last finish
                # never stalls the DVE queue
                while fill_proj:
                    fill_proj.popleft()[1]()
                for _ in range(4):
                    if fill_oproj:
                        fill_oproj.popleft()[1]()
                while pending:
                    pending.popleft()()
            while fill_oproj:
                fill_oproj.popleft()[1]()

    nc.compile()
    return nc


def _get_compiled():
    global _compiled
    if _compiled is None:
        _compiled = _build()
    return _compiled


def _host_inputs(x, wq, bq, wkv, bkv, wo):
    import jax.numpy as jnp

    def to_bf16(a):
        return np.asarray(jnp.asarray(a, dtype=jnp.bfloat16))

    pos = np.arange(T, dtype=np.float32)[:, None]
    i = np.arange(0, D, 2, dtype=np.float32)
    inv = np.exp(-(np.log(10000.0) * i / D))
    ang = pos * inv
    pe = np.zeros((T, D), np.float32)
    pe[:, 0::2] = np.sin(ang)
    pe[:, 1::2] = np.cos(ang)
    pet = np.ascontiguousarray(pe.T)                       # [D, T]

    # causal masks for the 4 diagonal tiles of a 512-wide tq block:
    # mask_j[p, c] = 1 if c >= 128*j + p
    c = np.arange(512)[None, :]
    p = np.arange(128)[:, None]
    msk = to_bf16(np.stack([(c >= 128 * j + p) for j in range(4)]).astype(np.float32))
    idb = to_bf16(np.eye(128, dtype=np.float32))

    xts = [to_bf16(np.ascontiguousarray(
        x[b].T.reshape(NE, 128, T).transpose(1, 0, 2))) for b in range(B)]
    in_maps = []
    for core in range(8):
        b, g = divmod(core, G)
        bq_g = bq[g * NQ:(g + 1) * NQ].reshape(QPG, D)     # [h, d]
        wqkv = np.concatenate(
            [wq[:, g * NQ:(g + 1) * NQ], wkv[:, g * NKV:(g + 1) * NKV]], axis=1)
        in_maps.append({
            "xt": xts[b],
            "wqkv": to_bf16(np.ascontiguousarray(
                wqkv.reshape(NE, 128, NQ + NKV).transpose(1, 0, 2))),
            "wo": to_bf16(wo[g * NQ:(g + 1) * NQ, :]),
            "pet": to_bf16(pet),
            "bq": np.ascontiguousarray(bq_g.T).astype(np.float32),
            "bk": np.ascontiguousarray(
                bkv[g * NKV:g * NKV + D].reshape(D, 1)).astype(np.float32),
            "bv": np.ascontiguousarray(
                bkv[g * NKV + D:(g + 1) * NKV].reshape(D, 1)).astype(np.float32),
            "msk": msk,
            "idb": idb,
        })
    return in_maps


def run(x, wq, bq, wkv, bkv, wo, trace=False):
    from concourse.bass_utils import run_bass_kernel_spmd

    nc = _get_compiled()
    in_maps = _host_inputs(
        np.asarray(x, np.float32), np.asarray(wq, np.float32),
        np.asarray(bq, np.float32), np.asarray(wkv, np.float32),
        np.asarray(bkv, np.float32), np.asarray(wo, np.float32),
    )
    res = run_bass_kernel_spmd(nc, in_maps, core_ids=list(range(8)), trace=trace)
    out = np.zeros((B, T, E), np.float32)
    for core in range(8):
        b = core // G
        out[b] += np.asarray(res.results[core]["out"], dtype=np.float32)
    return out, res


def kernel(x, wq, bq, wkv, bkv, wo):
    out, _ = run(x, wq, bq, wkv, bkv, wo, trace=False)
    return out
